# revision 16
# baseline (speedup 1.0000x reference)
"""AGF attention (graph-filter attention) distributed Bass kernel for 8 TRN2 cores.

Sharding: batch x head-pair (data + head parallel). Core i handles batch
b = i//4 and heads {2*(i%4), 2*(i%4)+1}. Each core computes its partial
output projection (summed over its 2 heads); a bf16 ReduceScatter over the
4 cores of each batch produces row shards of that batch's [N, D] output,
which the host concatenates.

v2 design (per core):
  - P = exp(S^T/8) for the CURRENT head is kept RESIDENT in SBUF
    ([128, NC, N] fp8 = 128 KB/partition) -- no HBM spill/reload. The three
    graph-filter applications read P straight from SBUF.
  - Production: S^T chunks via bf16 matmuls (KT chunk stationary) into
    [128, 2048] PSUM tiles (4 banks x 2 bufs), exp'd by ACT directly into
    P_sb as fp8. ACT is the kernel's bottleneck engine (~250 us of exp).
  - Applications: fp8 DoubleRow matmuls, stationary [t*16 | 1] (TW=80),
    moving P pairs, accumulate u^T = [16*A_u t | r]^T in a [80, NB, 512]
    PSUM tile (8 banks). u^T -> bf16 -> PE-transposed back to natural
    layout in packs of 8 chunks per PSUM bank; epilogue (normalize by 1/r,
    t-requantize to fp8, y accumulation) is BATCHED into a handful of
    full-size DVE instructions using stride-0 broadcast APs.
  - Head 1's Q^T/K^T are produced in setup and spilled to DRAM (bf16),
    reloaded into the same SBUF tiles after head 0's production.
  - Output projection: y (natural, f32, both heads) -> PE transpose ->
    po PSUM accumulates BOTH heads -> bf16 -> ReduceScatter over the
    4-core batch group.
"""

import numpy as np

import concourse.mybir as mybir
import concourse.tile as tile
from concourse import bacc
from concourse.bass import ds
from concourse.masks import make_identity

dt = mybir.dt
F32 = dt.float32
BF16 = dt.bfloat16
FP8 = dt.float8e4
AF = mybir.ActivationFunctionType
ALU = mybir.AluOpType
AX = mybir.AxisListType
DR = mybir.MatmulPerfMode.DoubleRow

D = 256      # model dim
DH = 64      # head dim
HPC = 2      # heads per core
LN_EPS = 1e-5
SM_SCALE = 0.125  # 1/sqrt(DH)
TW = 80      # t tile width: 64 t cols + 1 rowsum + pad to 16 (DoubleRow)
T_SCALE = 16.0


def build_kernel(nc, N=4096, replica_groups=((0, 1, 2, 3), (4, 5, 6, 7)),
                 p_dtype=FP8, collective=True):
    NC = N // 128          # 128-row chunks
    NB = N // 512          # 512-col blocks
    KD = D // 128          # 128-deep contraction chunks of the model dim
    replica_groups = [list(g) for g in replica_groups]

    G = len(replica_groups[0]) if collective else 4
    x = nc.dram_tensor("x", [N, D], F32, kind="ExternalInput")
    wq_d = nc.dram_tensor("wq", [D, HPC * DH], F32, kind="ExternalInput")
    wk_d = nc.dram_tensor("wk", [D, HPC * DH], F32, kind="ExternalInput")
    wv_d = nc.dram_tensor("wv", [D, HPC * DH], F32, kind="ExternalInput")
    wo_d = nc.dram_tensor("wo", [HPC * DH, D], F32, kind="ExternalInput")
    gamma_d = nc.dram_tensor("gamma2", [1, HPC * DH], F32, kind="ExternalInput")
    beta_d = nc.dram_tensor("beta2", [1, HPC * DH], F32, kind="ExternalInput")
    cf_d = nc.dram_tensor("cf", [1, HPC * 4], F32, kind="ExternalInput")
    out_d = nc.dram_tensor("out", [N // G, D], BF16, kind="ExternalOutput")

    with tile.TileContext(nc) as tc:
        _body(nc, tc, N, NC, NB, KD, replica_groups,
              x, wq_d, wk_d, wv_d, wo_d, gamma_d, beta_d, cf_d, out_d,
              collective)
    return nc


def _body(nc, tc, N, NC, NB, KD, replica_groups,
          x, wq_d, wk_d, wv_d, wo_d, gamma_d, beta_d, cf_d, out_d,
          collective=True):
    with (
        tc.tile_pool(name="persist", bufs=1) as pp,
        tc.tile_pool(name="dram", bufs=1, space="DRAM") as dram,
    ):
        # ---------- constants ----------
        ident = pp.tile([128, 128], F32)
        make_identity(nc, ident)
        ident_b = pp.tile([128, 128], BF16)
        nc.vector.tensor_copy(ident_b[:], ident[:])

        c1 = pp.tile([1, 8], F32)
        nc.sync.dma_start(c1[:], cf_d.ap())
        cbc = pp.tile([128, 8], F32)
        nc.gpsimd.partition_broadcast(cbc[:], c1[:])
        cbc16 = pp.tile([128, 8], F32)
        nc.vector.tensor_scalar_mul(cbc16[:], cbc[:], 1.0 / T_SCALE)

        # ---------- persistent work tiles ----------
        QT = pp.tile([64, N], BF16)     # current head's Q^T
        KT = pp.tile([64, N], BF16)     # current head's K^T
        vones = pp.tile([128, NC, HPC, TW], FP8)   # [v_ln | 1] app-1 lhs
        nc.vector.memset(vones[:, :, :, 64:TW], 1.0)
        vln = pp.tile([128, NC, HPC, 64], BF16)    # LayerNorm'd v
        tq = pp.tile([128, NC, TW], FP8)           # [16*t_k | 1] app lhs
        nc.vector.memset(tq[:, :, 64:TW], 1.0)
        uSt = pp.tile([128, NC, TW], BF16)         # transposed-back u chunks
        rinv = pp.tile([128, NC, 1], F32)          # 1/rowsum per (n-chunk)
        yt = pp.tile([128, NC, HPC, 64], F32)      # y, natural, both heads

        wo_s = pp.tile([64, HPC, 256], BF16)
        qk_dr = dram.tile([2, 64, N], BF16)        # head-1 Q^T/K^T spill
        bounce_in = dram.tile([N, D], BF16)

        # ================= setup =================
        with tc.tile_pool(name="setup", bufs=1) as sp:
            # weights -> SBUF bf16
            wst = sp.tile([128, 3, KD, 128], F32)
            nc.sync.dma_start(
                wst[:, 0], wq_d.ap().rearrange("(o p) m -> p o m", p=128))
            nc.sync.dma_start(
                wst[:, 1], wk_d.ap().rearrange("(o p) m -> p o m", p=128))
            nc.scalar.dma_start(
                wst[:, 2], wv_d.ap().rearrange("(o p) m -> p o m", p=128))
            wsb = sp.tile([128, 3, KD, 128], BF16)
            nc.vector.tensor_copy(wsb[:], wst[:])
            wq_s, wk_s, wv_s = wsb[:, 0], wsb[:, 1], wsb[:, 2]

            wo_f = sp.tile([64, HPC, 256], F32)
            nc.scalar.dma_start(
                wo_f[:], wo_d.ap().rearrange("(h d) m -> d h m", h=HPC))
            nc.vector.tensor_copy(wo_s[:], wo_f[:])

            g1 = sp.tile([1, 128], F32)
            nc.sync.dma_start(g1[:], gamma_d.ap())
            gbc = sp.tile([128, 1, HPC, 64], F32)
            nc.gpsimd.partition_broadcast(
                gbc.rearrange("p o a b -> p (o a b)"), g1[:])
            b1 = sp.tile([1, 128], F32)
            nc.sync.dma_start(b1[:], beta_d.ap())
            bbc = sp.tile([128, 1, HPC, 64], F32)
            nc.gpsimd.partition_broadcast(
                bbc.rearrange("p o a b -> p (o a b)"), b1[:])

            # x -> x^T via PE transposes (low latency; no DRAM bounce)
            xT = sp.tile([128, KD, N], BF16)
            with (
                tc.tile_pool(name="xsetup", bufs=3) as xp,
                tc.tile_pool(name="xt_psum", bufs=2, space="PSUM") as xtp,
            ):
                for cg in range(NC // 2):
                    if cg % 2 == 0:
                        xf = xp.tile([128, 4, D], F32, tag="xf")
                        nc.sync.dma_start(
                            xf[:],
                            x.ap().rearrange("(o p) d -> p o d", p=128)[
                                :, ds(cg * 2, 4), :])
                    tps = xtp.tile([128, 2, KD, 128], F32, tag="tx")
                    for j in range(2):
                        for kd in range(KD):
                            nc.tensor.transpose(
                                tps[:, j, kd, :],
                                xf[:, (cg % 2) * 2 + j, ds(kd * 128, 128)],
                                ident[:])
                    nc.vector.tensor_copy(
                        xT[:, :, ds(cg * 256, 256)].rearrange(
                            "p k (j c) -> p j k c", j=2), tps[:])

            # ---- Q^T/K^T (head 0 -> SBUF, head 1 -> DRAM) + V/LN ----
            vsb = sp.tile([128, NC, HPC, 64], BF16)
            s1 = sp.tile([128, NC, HPC], F32)
            sqs = sp.tile([128, NC, HPC, 64], BF16)
            s2 = sp.tile([128, NC, HPC], F32)
            mu = sp.tile([128, NC, HPC, 1], F32)
            var = sp.tile([128, NC, HPC, 1], F32)
            rstd = sp.tile([128, NC, HPC, 1], F32)
            with (
                tc.tile_pool(name="qk_psum", bufs=2, space="PSUM") as qpp,
                tc.tile_pool(name="v_psum", bufs=2, space="PSUM") as vpp,
                tc.tile_pool(name="qk_st", bufs=2) as qst,
            ):
                def emit_qk(h):
                    for qi, w_s in ((0, wq_s), (1, wk_s)):
                        for nb in range(N // 1024):
                            ps = qpp.tile([64, 1024], F32, tag="psq")
                            for s in range(2):
                                for kd in range(KD):
                                    nc.tensor.matmul(
                                        ps[:, ds(s * 512, 512)],
                                        w_s[:, kd, ds(h * 64, 64)],
                                        xT[:, kd,
                                           ds(nb * 1024 + s * 512, 512)],
                                        start=(kd == 0), stop=(kd == KD - 1))
                            if h == 0:
                                dst = QT if qi == 0 else KT
                                nc.vector.tensor_copy(
                                    dst[:, ds(nb * 1024, 1024)], ps[:])
                            else:
                                stg = qst.tile([64, 1024], BF16, tag="stg")
                                nc.vector.tensor_copy(stg[:], ps[:])
                                nc.scalar.dma_start(
                                    qk_dr[qi, :, ds(nb * 1024, 1024)], stg[:])

                emit_qk(0)
                # V projection: 4 chunks per PSUM bank
                for cg in range(NC // 4):
                    vps = vpp.tile([128, 4, 128], F32, tag="vps")
                    for j in range(4):
                        for kd in range(KD):
                            nc.tensor.matmul(
                                vps[:, j, :],
                                xT[:, kd, ds((cg * 4 + j) * 128, 128)],
                                wv_s[:, kd, :],
                                start=(kd == 0), stop=(kd == KD - 1))
                    nc.vector.tensor_copy(
                        vsb[:, ds(cg * 4, 4), :, :],
                        vps[:].rearrange("p j (h d) -> p j h d", h=HPC))
                emit_qk(1)

            # ---- batched LayerNorm over dim_head ----
            nc.vector.tensor_reduce(
                s1.rearrange("p a b -> p (a b)"), vsb[:], axis=AX.X,
                op=ALU.add)
            nc.vector.tensor_tensor(sqs[:], vsb[:], vsb[:], ALU.mult)
            nc.vector.tensor_reduce(
                s2.rearrange("p a b -> p (a b)"), sqs[:], axis=AX.X,
                op=ALU.add)
            muf = mu.rearrange("p a b c -> p (a b c)")
            varf = var.rearrange("p a b c -> p (a b c)")
            s1f = s1.rearrange("p a b -> p (a b)")
            s2f = s2.rearrange("p a b -> p (a b)")
            nc.vector.tensor_scalar_mul(muf, s1f, 1.0 / 64.0)
            # var = s2/64 - mu^2   (as (s2*(1/64) - mu) ... need mu^2)
            nc.vector.scalar_tensor_tensor(
                varf, muf, -1.0, muf, ALU.mult, ALU.mult)   # -mu^2
            nc.vector.scalar_tensor_tensor(
                varf, s2f, 1.0 / 64.0, varf, ALU.mult, ALU.add)
            nc.vector.tensor_scalar_add(varf, varf, LN_EPS)
            # rstd = exp(-0.5 * ln(var + eps))
            nc.scalar.activation(varf, varf, AF.Ln)
            nc.scalar.activation(
                rstd.rearrange("p a b c -> p (a b c)"), varf,
                AF.Exp, scale=-0.5)
            # vln = (vsb - mu) * rstd * gamma + beta   (broadcast APs)
            mu_b = mu[:].broadcast_to([128, NC, HPC, 64])
            rstd_b = rstd[:].broadcast_to([128, NC, HPC, 64])
            gb = gbc[:].broadcast_to([128, NC, HPC, 64])
            bb = bbc[:].broadcast_to([128, NC, HPC, 64])
            nc.vector.tensor_tensor(vln[:], vsb[:], mu_b, ALU.subtract)
            nc.vector.tensor_tensor(vln[:], vln[:], rstd_b, ALU.mult)
            nc.vector.tensor_tensor(vln[:], vln[:], gb, ALU.mult)
            nc.vector.tensor_tensor(vln[:], vln[:], bb, ALU.add)
            nc.vector.tensor_copy(vones[:, :, :, 0:64], vln[:])

        # ================= main: per-head production + applications =========
        NSPILL = 20                 # head-1 P chunks spilled to DRAM
        with tc.tile_pool(name="pmain", bufs=1) as pm:
            P_sb = pm.tile([128, NC, N], FP8)
            P_dr = dram.tile([NSPILL, 128, N], FP8)

            def app_lhs(h, app, mp):
                return (vones[:, ds(2 * mp, 2), h, :] if app == 1
                        else tq[:, ds(2 * mp, 2), :])

            def emit_app_mms(h, app, apo, uTs, nbp):
                """acc matmuls + copy-out, in NB//nbp n-passes. Yields
                after every mp-group so prod-h1 can interleave."""
                for ip in range(NB // nbp):
                    acc = apo.tile([TW, nbp, 512], F32, tag="acc")
                    for mp in range(NC // 2):
                        lhs = app_lhs(h, app, mp)
                        for j in range(nbp):
                            nc.tensor.matmul(
                                acc[:, j, :], lhs,
                                P_sb[:, ds(2 * mp, 2),
                                     ds((ip * nbp + j) * 512, 512)],
                                start=(mp == 0), stop=(mp == NC // 2 - 1),
                                perf_mode=DR)
                        yield
                    nc.vector.tensor_copy(
                        uTs.rearrange("p (b n) -> p b n", n=512)[
                            :, ds(ip * nbp, nbp), :], acc[:])
                    yield

            def emit_app_tail(h, app, trp, uTs):
                """transpose back (8 chunks/bank) + batched epilogue."""
                for pk in range(NC // 8):
                    tp = trp.tile([128, 8, TW], BF16, tag="tp")
                    for j in range(8):
                        nc.tensor.transpose(
                            tp[:, j, :],
                            uTs[:, ds((pk * 8 + j) * 128, 128)],
                            ident_b[0:TW, 0:TW])
                    nc.vector.tensor_copy(uSt[:, ds(pk * 8, 8), :], tp[:])
                    yield
                if app == 1:
                    nc.vector.reciprocal(rinv[:], uSt[:, :, 64:65])
                rb = rinv[:].broadcast_to([128, NC, 64])
                # tq = (uSt * scale) * (1/r)  [fp8, feeds next app]
                nc.vector.scalar_tensor_tensor(
                    tq[:, :, 0:64], uSt[:, :, 0:64],
                    T_SCALE if app == 1 else 1.0, rb, ALU.mult, ALU.mult)
                yh = yt[:, :, h, :]
                if app == 1:
                    nc.vector.tensor_scalar_mul(
                        yh, vln[:, :, h, :], cbc[:, ds(h * 4, 1)])
                # y += (c_k/16) * tq
                nc.vector.scalar_tensor_tensor(
                    yh, tq[:, :, 0:64], cbc16[:, ds(h * 4 + app, 1)],
                    yh, ALU.mult, ALU.add)
                yield

            # ---- head 0 production: full SBUF residency, 8-bank PSUM ----
            with tc.tile_pool(name="prod_psum", bufs=2, space="PSUM") as ppp:
                for mc in range(NC):
                    for half in range(2):
                        ps = ppp.tile([128, 2048], F32, tag="s")
                        for q in range(4):
                            nc.tensor.matmul(
                                ps[:, ds(q * 512, 512)],
                                KT[:, ds(mc * 128, 128)],
                                QT[:, ds(half * 2048 + q * 512, 512)],
                                start=True, stop=True)
                        nc.scalar.activation(
                            P_sb[:, mc, ds(half * 2048, 2048)], ps[:],
                            AF.Exp, scale=SM_SCALE)

            # head-1 Q^T/K^T reload (waits on prod-h0's last reads)
            nc.sync.dma_start(QT[:], qk_dr[0])
            nc.sync.dma_start(KT[:], qk_dr[1])

            # ---- overlap: apps h0 (PE/DVE) || production h1 (ACT) ----
            # PSUM: prod 2x[128,1024]=4 banks, acc [80,2,512]=2, tr 2.
            with (
                tc.tile_pool(name="ov_prod", bufs=2, space="PSUM") as ovp,
                tc.tile_pool(name="ov_acc", bufs=1, space="PSUM") as apo2,
                tc.tile_pool(name="ov_tr", bufs=2, space="PSUM") as trp2,
                tc.tile_pool(name="ov_sb", bufs=1) as ovs,
                tc.tile_pool(name="spill_sb", bufs=2) as sps,
            ):
                uTs0 = ovs.tile([TW, N], BF16)

                def prod1_unit(mc, qtr, stg):
                    """one quarter-chunk: 2 S-matmuls + exp."""
                    ps = ovp.tile([128, 1024], F32, tag="s1")
                    for q in range(2):
                        nc.tensor.matmul(
                            ps[:, ds(q * 512, 512)],
                            KT[:, ds(mc * 128, 128)],
                            QT[:, ds(qtr * 1024 + q * 512, 512)],
                            start=True, stop=True)
                    dst = (stg[:, ds(qtr * 1024, 1024)] if stg is not None
                           else P_sb[:, mc, ds(qtr * 1024, 1024)])
                    nc.scalar.activation(dst, ps[:], AF.Exp, scale=SM_SCALE)

                def prod1_spill_units():
                    for mc in range(NSPILL):
                        stg = sps.tile([128, N], FP8, tag="pt")
                        for qtr in range(4):
                            prod1_unit(mc, qtr, stg)
                            yield
                        nc.scalar.dma_start(P_dr[mc], stg[:])

                def apps0_units():
                    for app in range(1, 4):
                        yield from emit_app_mms(0, app, apo2, uTs0, nbp=2)
                        yield from emit_app_tail(0, app, trp2, uTs0)

                gp = prod1_spill_units()
                ga = apps0_units()
                done_p = done_a = False
                while not (done_p and done_a):
                    if not done_p:
                        done_p = next(gp, "end") == "end"
                    if not done_a:
                        for _ in range(3):
                            if next(ga, "end") == "end":
                                done_a = True
                                break
                # reload spilled chunks into P_sb (slots free after apps
                # h0; DMAs run during prod-h1's direct tail)
                for p in range(NSPILL // 2):
                    nc.sync.dma_start(
                        P_sb[:, ds(2 * p, 2), :],
                        P_dr[ds(2 * p, 2)].rearrange("c p n -> p c n"))
                # direct-to-SBUF tail of prod h1 (slots freed by apps h0)
                for mc in range(NSPILL, NC):
                    for qtr in range(4):
                        prod1_unit(mc, qtr, None)
                # app1 of head 1: 2-bank accumulator so its early pairs can
                # run under prod-h1's trailing exps
                for _ in emit_app_mms(1, 1, apo2, uTs0, nbp=2):
                    pass
                for _ in emit_app_tail(1, 1, trp2, uTs0):
                    pass

            # ---- apps 2,3 of head 1 (full 8-bank accumulator) ----
            for app in range(2, 4):
                with tc.tile_pool(name="uts", bufs=1) as up:
                    uTs = up.tile([TW, N], BF16)
                    with tc.tile_pool(name="acc_psum", bufs=1,
                                      space="PSUM") as apo:
                        for _ in emit_app_mms(1, app, apo, uTs, nbp=NB):
                            pass
                    with tc.tile_pool(name="tr_psum", bufs=2,
                                      space="PSUM") as trp:
                        for _ in emit_app_tail(1, app, trp, uTs):
                            pass

            # ---- output projection (both heads fused in PSUM) ----
            with (
                tc.tile_pool(name="ty_psum", bufs=2, space="PSUM") as typ,
                tc.tile_pool(name="o_psum", bufs=2, space="PSUM") as opp,
                tc.tile_pool(name="o_st", bufs=3) as ost,
            ):
                for ci in range(NC):
                    ty = typ.tile([64, HPC, 128], F32, tag="ty")
                    for h in range(HPC):
                        nc.tensor.transpose(
                            ty[:, h, :], yt[:, ci, h, :], ident[:])
                    yst = ost.tile([64, HPC, 128], BF16, tag="yst")
                    nc.vector.tensor_copy(yst[:], ty[:])
                    po = opp.tile([128, 256], F32, tag="po")
                    for h in range(HPC):
                        nc.tensor.matmul(
                            po[:], yst[:, h, :], wo_s[:, h, :],
                            start=(h == 0), stop=(h == HPC - 1))
                    ob = ost.tile([128, 256], BF16, tag="ob")
                    nc.vector.tensor_copy(ob[:], po[:])
                    nc.sync.dma_start(
                        bounce_in.rearrange("(c p) d -> p c d", p=128)[
                            :, ci, :], ob[:])

        # ---------- ReduceScatter (bf16) over the batch group + output ------
        if not collective:
            nc.sync.dma_start(
                out_d.ap().rearrange("(c p) d -> p c d", p=128),
                bounce_in.rearrange("(c p) d -> p c d", p=128)[
                    :, 0:NC // 4, :])
            return
        G = len(replica_groups[0])
        bounce_out = dram.tile([N // G, D], BF16)
        nc.gpsimd.collective_compute(
            "ReduceScatter", ALU.add, replica_groups=replica_groups,
            ins=[bounce_in.opt()], outs=[bounce_out.opt()])
        nc.sync.dma_start(out_d.ap(), bounce_out[:])


# ----------------------------------------------------------------------------
# host-side entry point
# ----------------------------------------------------------------------------

_CACHED = {}


def _get_compiled(N=4096, n_cores=8, p_dtype=FP8):
    key = (N, n_cores, p_dtype)
    if key not in _CACHED:
        groups = [list(range(g * 4, g * 4 + 4)) for g in range(2)] \
            if n_cores == 8 else [list(range(n_cores))]
        nc = bacc.Bacc("TRN2", target_bir_lowering=False, debug=False,
                       num_devices=n_cores)
        build_kernel(nc, N=N, replica_groups=groups, p_dtype=p_dtype)
        nc.compile()
        _CACHED[key] = nc
    return _CACHED[key]


def make_in_maps(x, Wq, Wk, Wv, Wo, gamma, beta, coeffs, n_cores=8):
    """Shard full inputs into per-core input maps (batch + head-pair)."""
    x = np.asarray(x, np.float32)
    Wq = np.asarray(Wq, np.float32)
    Wk = np.asarray(Wk, np.float32)
    Wv = np.asarray(Wv, np.float32)
    Wo = np.asarray(Wo, np.float32)
    gamma = np.asarray(gamma, np.float32)
    beta = np.asarray(beta, np.float32)
    coeffs = np.asarray(coeffs, np.float32)
    g2 = np.concatenate([gamma, gamma]).reshape(1, 128).copy()
    b2 = np.concatenate([beta, beta]).reshape(1, 128).copy()
    in_maps = []
    for core in range(n_cores):
        b = core // 4 if n_cores == 8 else 0
        hp = core % 4 if n_cores == 8 else core
        cols = slice(hp * 128, (hp + 1) * 128)
        in_maps.append({
            "x": np.ascontiguousarray(x[b]),
            "wq": np.ascontiguousarray(Wq[:, cols]),
            "wk": np.ascontiguousarray(Wk[:, cols]),
            "wv": np.ascontiguousarray(Wv[:, cols]),
            "wo": np.ascontiguousarray(Wo[cols, :]),
            "gamma2": g2,
            "beta2": b2,
            "cf": np.ascontiguousarray(coeffs[2 * hp: 2 * hp + 2].reshape(1, 8)),
        })
    return in_maps


def kernel(x, Wq, Wk, Wv, Wo, gamma, beta, coeffs, trace=False):
    from concourse.bass_utils import run_bass_kernel_spmd

    n_cores = 8
    nc = _get_compiled(4096, n_cores)
    in_maps = make_in_maps(x, Wq, Wk, Wv, Wo, gamma, beta, coeffs, n_cores)
    res = run_bass_kernel_spmd(nc, in_maps, core_ids=list(range(n_cores)),
                               trace=trace)
    # each core returns its ReduceScatter shard: rank k of a batch group
    # holds rows [k*N/4, (k+1)*N/4) of that batch's output
    N = 4096
    out = np.empty((2, N, 256), np.float32)
    for b, cores in enumerate([[0, 1, 2, 3], [4, 5, 6, 7]]):
        for k, c in enumerate(cores):
            shard = np.asarray(res.results[c]["out"]).astype(np.float32)
            out[b, k * (N // 4):(k + 1) * (N // 4)] = shard
    if trace:
        kernel.last_result = res
    return out


# revision 29
# speedup vs baseline: 1.0196x; 1.0196x over previous
"""AGF attention (graph-filter attention) distributed Bass kernel for 8 TRN2 cores.

Sharding: batch x head-pair (data + head parallel). Core i handles batch
b = i//4 and heads {2*(i%4), 2*(i%4)+1}. Each core computes its partial
output projection (summed over its 2 heads); a bf16 ReduceScatter over the
4 cores of each batch produces row shards of that batch's [N, D] output,
which the host concatenates.

v2 design (per core):
  - P = exp(S^T/8) for the CURRENT head is kept RESIDENT in SBUF
    ([128, NC, N] fp8 = 128 KB/partition) -- no HBM spill/reload. The three
    graph-filter applications read P straight from SBUF.
  - Production: S^T chunks via bf16 matmuls (KT chunk stationary) into
    [128, 2048] PSUM tiles (4 banks x 2 bufs), exp'd by ACT directly into
    P_sb as fp8. ACT is the kernel's bottleneck engine (~250 us of exp).
  - Applications: fp8 DoubleRow matmuls, stationary [t*16 | 1] (TW=80),
    moving P pairs, accumulate u^T = [16*A_u t | r]^T in a [80, NB, 512]
    PSUM tile (8 banks). u^T -> bf16 -> PE-transposed back to natural
    layout in packs of 8 chunks per PSUM bank; epilogue (normalize by 1/r,
    t-requantize to fp8, y accumulation) is BATCHED into a handful of
    full-size DVE instructions using stride-0 broadcast APs.
  - Head 1's Q^T/K^T are produced in setup and spilled to DRAM (bf16),
    reloaded into the same SBUF tiles after head 0's production.
  - Output projection: y (natural, f32, both heads) -> PE transpose ->
    po PSUM accumulates BOTH heads -> bf16 -> ReduceScatter over the
    4-core batch group.
"""

import numpy as np

import concourse.mybir as mybir
import concourse.tile as tile
from concourse import bacc
from concourse.bass import ds
from concourse.masks import make_identity

dt = mybir.dt
F32 = dt.float32
BF16 = dt.bfloat16
FP8 = dt.float8e4
AF = mybir.ActivationFunctionType
ALU = mybir.AluOpType
AX = mybir.AxisListType
DR = mybir.MatmulPerfMode.DoubleRow

D = 256      # model dim
DH = 64      # head dim
HPC = 2      # heads per core
LN_EPS = 1e-5
SM_SCALE = 0.125  # 1/sqrt(DH)
TW = 80      # t tile width: 64 t cols + 1 rowsum + pad to 16 (DoubleRow)
T_SCALE = 16.0


def build_kernel(nc, N=4096, replica_groups=((0, 1, 2, 3), (4, 5, 6, 7)),
                 p_dtype=FP8, collective=True):
    NC = N // 128          # 128-row chunks
    NB = N // 512          # 512-col blocks
    KD = D // 128          # 128-deep contraction chunks of the model dim
    replica_groups = [list(g) for g in replica_groups]

    G = len(replica_groups[0]) if collective else 4
    x = nc.dram_tensor("x", [N, D], F32, kind="ExternalInput")
    wq_d = nc.dram_tensor("wq", [D, HPC * DH], F32, kind="ExternalInput")
    wk_d = nc.dram_tensor("wk", [D, HPC * DH], F32, kind="ExternalInput")
    wv_d = nc.dram_tensor("wv", [D, HPC * DH], F32, kind="ExternalInput")
    wo_d = nc.dram_tensor("wo", [HPC * DH, D], F32, kind="ExternalInput")
    gamma_d = nc.dram_tensor("gamma2", [1, HPC * DH], F32, kind="ExternalInput")
    beta_d = nc.dram_tensor("beta2", [1, HPC * DH], F32, kind="ExternalInput")
    cf_d = nc.dram_tensor("cf", [1, HPC * 4], F32, kind="ExternalInput")
    out_d = nc.dram_tensor("out", [N // G, D], BF16, kind="ExternalOutput")

    with tile.TileContext(nc) as tc:
        _body(nc, tc, N, NC, NB, KD, replica_groups,
              x, wq_d, wk_d, wv_d, wo_d, gamma_d, beta_d, cf_d, out_d,
              collective)
    return nc


def _body(nc, tc, N, NC, NB, KD, replica_groups,
          x, wq_d, wk_d, wv_d, wo_d, gamma_d, beta_d, cf_d, out_d,
          collective=True):
    with (
        tc.tile_pool(name="persist", bufs=1) as pp,
        tc.tile_pool(name="dram", bufs=1, space="DRAM") as dram,
    ):
        # ---------- constants ----------
        ident = pp.tile([128, 128], F32)
        make_identity(nc, ident)
        ident_b = pp.tile([128, 128], BF16)
        nc.vector.tensor_copy(ident_b[:], ident[:])

        c1 = pp.tile([1, 8], F32)
        nc.sync.dma_start(c1[:], cf_d.ap())
        cbc = pp.tile([128, 8], F32)
        nc.gpsimd.partition_broadcast(cbc[:], c1[:])
        cbc16 = pp.tile([128, 8], F32)
        nc.vector.tensor_scalar_mul(cbc16[:], cbc[:], 1.0 / T_SCALE)

        # ---------- persistent work tiles ----------
        QT = pp.tile([64, N], BF16)     # current head's Q^T
        KT = pp.tile([64, N], BF16)     # current head's K^T
        vones = pp.tile([128, NC, HPC, TW], FP8)   # [v_ln | 1] app-1 lhs
        nc.vector.memset(vones[:, :, :, 64:TW], 1.0)
        vln = pp.tile([128, NC, HPC, 64], BF16)    # LayerNorm'd v
        tq = pp.tile([128, NC, TW], FP8)           # [16*t_k | 1] app lhs
        nc.vector.memset(tq[:, :, 64:TW], 1.0)
        uSt = pp.tile([128, NC, TW], BF16)         # transposed-back u chunks
        rinv = pp.tile([128, NC, 1], F32)          # 1/rowsum per (n-chunk)
        yt = pp.tile([128, NC, HPC, 64], F32)      # y, natural, both heads

        wo_s = pp.tile([64, HPC, 256], BF16)
        qk_dr = dram.tile([2, 64, N], BF16)        # head-1 Q^T/K^T spill
        bounce_in = dram.tile([N, D], BF16)

        # ================= setup =================
        with tc.tile_pool(name="setup", bufs=1) as sp:
            # weights -> SBUF bf16
            wst = sp.tile([128, 3, KD, 128], F32)
            nc.sync.dma_start(
                wst[:, 0], wq_d.ap().rearrange("(o p) m -> p o m", p=128))
            nc.sync.dma_start(
                wst[:, 1], wk_d.ap().rearrange("(o p) m -> p o m", p=128))
            nc.scalar.dma_start(
                wst[:, 2], wv_d.ap().rearrange("(o p) m -> p o m", p=128))
            wsb = sp.tile([128, 3, KD, 128], BF16)
            nc.vector.tensor_copy(wsb[:], wst[:])
            wq_s, wk_s, wv_s = wsb[:, 0], wsb[:, 1], wsb[:, 2]

            wo_f = sp.tile([64, HPC, 256], F32)
            nc.scalar.dma_start(
                wo_f[:], wo_d.ap().rearrange("(h d) m -> d h m", h=HPC))
            nc.vector.tensor_copy(wo_s[:], wo_f[:])

            g1 = sp.tile([1, 128], F32)
            nc.sync.dma_start(g1[:], gamma_d.ap())
            gbc = sp.tile([128, 1, HPC, 64], F32)
            nc.gpsimd.partition_broadcast(
                gbc.rearrange("p o a b -> p (o a b)"), g1[:])
            b1 = sp.tile([1, 128], F32)
            nc.sync.dma_start(b1[:], beta_d.ap())
            bbc = sp.tile([128, 1, HPC, 64], F32)
            nc.gpsimd.partition_broadcast(
                bbc.rearrange("p o a b -> p (o a b)"), b1[:])

            # x -> x^T via PE transposes (low latency; no DRAM bounce)
            xT = sp.tile([128, KD, N], BF16)
            with (
                tc.tile_pool(name="xsetup", bufs=3) as xp,
                tc.tile_pool(name="xt_psum", bufs=2, space="PSUM") as xtp,
            ):
                for cg in range(NC // 2):
                    if cg % 2 == 0:
                        xf = xp.tile([128, 4, D], F32, tag="xf")
                        nc.sync.dma_start(
                            xf[:],
                            x.ap().rearrange("(o p) d -> p o d", p=128)[
                                :, ds(cg * 2, 4), :])
                    tps = xtp.tile([128, 2, KD, 128], F32, tag="tx")
                    for j in range(2):
                        for kd in range(KD):
                            nc.tensor.transpose(
                                tps[:, j, kd, :],
                                xf[:, (cg % 2) * 2 + j, ds(kd * 128, 128)],
                                ident[:])
                    # ACT is idle through setup; keep copies off the DVE
                    # critical chain that gates the LayerNorm activations
                    nc.scalar.activation(
                        xT[:, :, ds(cg * 256, 256)].rearrange(
                            "p k (j c) -> p j k c", j=2), tps[:], AF.Copy)

            # ---- Q^T/K^T (head 0 -> SBUF, head 1 -> DRAM) + V/LN ----
            vsb = sp.tile([128, NC, HPC, 64], BF16)
            s1 = sp.tile([128, NC, HPC], F32)
            sqs = sp.tile([128, NC, HPC, 64], BF16)
            s2 = sp.tile([128, NC, HPC], F32)
            mu = sp.tile([128, NC, HPC, 1], F32)
            var = sp.tile([128, NC, HPC, 1], F32)
            rstd = sp.tile([128, NC, HPC, 1], F32)
            with (
                tc.tile_pool(name="qk_psum", bufs=2, space="PSUM") as qpp,
                tc.tile_pool(name="v_psum", bufs=2, space="PSUM") as vpp,
                tc.tile_pool(name="qk_st", bufs=2) as qst,
            ):
                def emit_qk(h):
                    for qi, w_s in ((0, wq_s), (1, wk_s)):
                        for nb in range(N // 1024):
                            ps = qpp.tile([64, 1024], F32, tag="psq")
                            for s in range(2):
                                for kd in range(KD):
                                    nc.tensor.matmul(
                                        ps[:, ds(s * 512, 512)],
                                        w_s[:, kd, ds(h * 64, 64)],
                                        xT[:, kd,
                                           ds(nb * 1024 + s * 512, 512)],
                                        start=(kd == 0), stop=(kd == KD - 1))
                            if h == 0:
                                dst = QT if qi == 0 else KT
                                nc.scalar.activation(
                                    dst[:, ds(nb * 1024, 1024)], ps[:],
                                    AF.Copy)
                            else:
                                stg = qst.tile([64, 1024], BF16, tag="stg")
                                nc.vector.tensor_copy(stg[:], ps[:])
                                nc.scalar.dma_start(
                                    qk_dr[qi, :, ds(nb * 1024, 1024)], stg[:])

                emit_qk(0)
                # V projection: 4 chunks per PSUM bank
                for cg in range(NC // 4):
                    vps = vpp.tile([128, 4, 128], F32, tag="vps")
                    for j in range(4):
                        for kd in range(KD):
                            nc.tensor.matmul(
                                vps[:, j, :],
                                xT[:, kd, ds((cg * 4 + j) * 128, 128)],
                                wv_s[:, kd, :],
                                start=(kd == 0), stop=(kd == KD - 1))
                    nc.vector.tensor_copy(
                        vsb[:, ds(cg * 4, 4), :, :],
                        vps[:].rearrange("p j (h d) -> p j h d", h=HPC))
                emit_qk(1)

            # ---- batched LayerNorm over dim_head ----
            nc.vector.tensor_reduce(
                s1.rearrange("p a b -> p (a b)"), vsb[:], axis=AX.X,
                op=ALU.add)
            nc.vector.tensor_tensor(sqs[:], vsb[:], vsb[:], ALU.mult)
            nc.vector.tensor_reduce(
                s2.rearrange("p a b -> p (a b)"), sqs[:], axis=AX.X,
                op=ALU.add)
            muf = mu.rearrange("p a b c -> p (a b c)")
            varf = var.rearrange("p a b c -> p (a b c)")
            s1f = s1.rearrange("p a b -> p (a b)")
            s2f = s2.rearrange("p a b -> p (a b)")
            nc.vector.tensor_scalar_mul(muf, s1f, 1.0 / 64.0)
            # var = s2/64 - mu^2   (as (s2*(1/64) - mu) ... need mu^2)
            nc.vector.scalar_tensor_tensor(
                varf, muf, -1.0, muf, ALU.mult, ALU.mult)   # -mu^2
            nc.vector.scalar_tensor_tensor(
                varf, s2f, 1.0 / 64.0, varf, ALU.mult, ALU.add)
            nc.vector.tensor_scalar_add(varf, varf, LN_EPS)
            # rstd = exp(-0.5 * ln(var + eps))
            nc.scalar.activation(varf, varf, AF.Ln)
            nc.scalar.activation(
                rstd.rearrange("p a b c -> p (a b c)"), varf,
                AF.Exp, scale=-0.5)
            # vln = (vsb - mu) * rstd * gamma + beta   (broadcast APs)
            mu_b = mu[:].broadcast_to([128, NC, HPC, 64])
            rstd_b = rstd[:].broadcast_to([128, NC, HPC, 64])
            gb = gbc[:].broadcast_to([128, NC, HPC, 64])
            bb = bbc[:].broadcast_to([128, NC, HPC, 64])
            nc.vector.tensor_tensor(vln[:], vsb[:], mu_b, ALU.subtract)
            nc.vector.tensor_tensor(vln[:], vln[:], rstd_b, ALU.mult)
            nc.vector.tensor_tensor(vln[:], vln[:], gb, ALU.mult)
            nc.vector.tensor_tensor(vln[:], vln[:], bb, ALU.add)
            nc.vector.tensor_copy(vones[:, :, :, 0:64], vln[:])

        # ================= main: per-head production + applications =========
        NSPILL = 20                 # head-1 P chunks spilled to DRAM
        with tc.tile_pool(name="pmain", bufs=1) as pm:
            P_sb = pm.tile([128, NC, N], FP8)
            P_dr = dram.tile([NSPILL, 128, N], FP8)

            def app_lhs(h, app, mp):
                return (vones[:, ds(2 * mp, 2), h, :] if app == 1
                        else tq[:, ds(2 * mp, 2), :])

            def emit_app_mms(h, app, apo, uTs, nbp):
                """acc matmuls + copy-out, in NB//nbp n-passes. Yields
                after every mp-group so prod-h1 can interleave."""
                for ip in range(NB // nbp):
                    acc = apo.tile([TW, nbp, 512], F32, tag="acc")
                    for mp in range(NC // 2):
                        lhs = app_lhs(h, app, mp)
                        for j in range(nbp):
                            nc.tensor.matmul(
                                acc[:, j, :], lhs,
                                P_sb[:, ds(2 * mp, 2),
                                     ds((ip * nbp + j) * 512, 512)],
                                start=(mp == 0), stop=(mp == NC // 2 - 1),
                                perf_mode=DR)
                        yield
                    nc.vector.tensor_copy(
                        uTs.rearrange("p (b n) -> p b n", n=512)[
                            :, ds(ip * nbp, nbp), :], acc[:])
                    yield

            def emit_app_tail(h, app, trp, uTs):
                """transpose back (8 chunks/bank) + batched epilogue."""
                for pk in range(NC // 8):
                    tp = trp.tile([128, 8, TW], BF16, tag="tp")
                    for j in range(8):
                        nc.tensor.transpose(
                            tp[:, j, :],
                            uTs[:, ds((pk * 8 + j) * 128, 128)],
                            ident_b[0:TW, 0:TW])
                    nc.vector.tensor_copy(uSt[:, ds(pk * 8, 8), :], tp[:])
                    yield
                if app == 1:
                    nc.vector.reciprocal(rinv[:], uSt[:, :, 64:65])
                rb = rinv[:].broadcast_to([128, NC, 64])
                # tq = (uSt * scale) * (1/r)  [fp8, feeds next app]
                nc.vector.scalar_tensor_tensor(
                    tq[:, :, 0:64], uSt[:, :, 0:64],
                    T_SCALE if app == 1 else 1.0, rb, ALU.mult, ALU.mult)
                yh = yt[:, :, h, :]
                if app == 1:
                    nc.vector.tensor_scalar_mul(
                        yh, vln[:, :, h, :], cbc[:, ds(h * 4, 1)])
                # y += (c_k/16) * tq
                nc.vector.scalar_tensor_tensor(
                    yh, tq[:, :, 0:64], cbc16[:, ds(h * 4 + app, 1)],
                    yh, ALU.mult, ALU.add)
                yield

            # ---- head 0 production: full SBUF residency, 8-bank PSUM ----
            with tc.tile_pool(name="prod_psum", bufs=2, space="PSUM") as ppp:
                for mc in range(NC):
                    for half in range(2):
                        ps = ppp.tile([128, 2048], F32, tag="s")
                        for q in range(4):
                            nc.tensor.matmul(
                                ps[:, ds(q * 512, 512)],
                                KT[:, ds(mc * 128, 128)],
                                QT[:, ds(half * 2048 + q * 512, 512)],
                                start=True, stop=True)
                        nc.scalar.activation(
                            P_sb[:, mc, ds(half * 2048, 2048)], ps[:],
                            AF.Exp, scale=SM_SCALE)

            # head-1 Q^T/K^T reload (waits on prod-h0's last reads)
            nc.sync.dma_start(QT[:], qk_dr[0])
            nc.sync.dma_start(KT[:], qk_dr[1])

            # ---- overlap: apps h0 (PE/DVE) || production h1 (ACT) ----
            # PSUM: prod 2x[128,1024]=4 banks, acc [80,2,512]=2, tr 2.
            with (
                tc.tile_pool(name="ov_prod", bufs=2, space="PSUM") as ovp,
                tc.tile_pool(name="ov_acc", bufs=1, space="PSUM") as apo2,
                tc.tile_pool(name="ov_tr", bufs=2, space="PSUM") as trp2,
                tc.tile_pool(name="ov_sb", bufs=1) as ovs,
                tc.tile_pool(name="spill_sb", bufs=2) as sps,
            ):
                uTs0 = ovs.tile([TW, N], BF16)

                def prod1_unit(mc, qtr, stg):
                    """one quarter-chunk: 2 S-matmuls + exp."""
                    ps = ovp.tile([128, 1024], F32, tag="s1")
                    for q in range(2):
                        nc.tensor.matmul(
                            ps[:, ds(q * 512, 512)],
                            KT[:, ds(mc * 128, 128)],
                            QT[:, ds(qtr * 1024 + q * 512, 512)],
                            start=True, stop=True)
                    dst = (stg[:, ds(qtr * 1024, 1024)] if stg is not None
                           else P_sb[:, mc, ds(qtr * 1024, 1024)])
                    nc.scalar.activation(dst, ps[:], AF.Exp, scale=SM_SCALE)

                def prod1_spill_units():
                    for mc in range(NSPILL):
                        stg = sps.tile([128, N], FP8, tag="pt")
                        for qtr in range(4):
                            prod1_unit(mc, qtr, stg)
                            yield
                        nc.scalar.dma_start(P_dr[mc], stg[:])

                def apps0_units():
                    for app in range(1, 4):
                        yield from emit_app_mms(0, app, apo2, uTs0, nbp=2)
                        yield from emit_app_tail(0, app, trp2, uTs0)

                gp = prod1_spill_units()
                ga = apps0_units()
                done_p = done_a = False
                while not (done_p and done_a):
                    if not done_p:
                        done_p = next(gp, "end") == "end"
                    if not done_a:
                        for _ in range(3):
                            if next(ga, "end") == "end":
                                done_a = True
                                break
                # reload spilled chunks into P_sb (slots free after apps
                # h0; DMAs run during prod-h1's direct tail)
                for p in range(NSPILL // 2):
                    nc.sync.dma_start(
                        P_sb[:, ds(2 * p, 2), :],
                        P_dr[ds(2 * p, 2)].rearrange("c p n -> p c n"))
                # direct-to-SBUF tail of prod h1 (slots freed by apps h0)
                for mc in range(NSPILL, NC):
                    for qtr in range(4):
                        prod1_unit(mc, qtr, None)
                # app1 of head 1: 2-bank accumulator so its early pairs can
                # run under prod-h1's trailing exps
                for _ in emit_app_mms(1, 1, apo2, uTs0, nbp=2):
                    pass
                for _ in emit_app_tail(1, 1, trp2, uTs0):
                    pass

            # ---- apps 2,3 of head 1 (full 8-bank accumulator) ----
            for app in range(2, 4):
                with tc.tile_pool(name="uts", bufs=1) as up:
                    uTs = up.tile([TW, N], BF16)
                    with tc.tile_pool(name="acc_psum", bufs=1,
                                      space="PSUM") as apo:
                        for _ in emit_app_mms(1, app, apo, uTs, nbp=NB):
                            pass
                    with tc.tile_pool(name="tr_psum", bufs=2,
                                      space="PSUM") as trp:
                        for _ in emit_app_tail(1, app, trp, uTs):
                            pass

            # ---- output projection (both heads fused in PSUM) ----
            with (
                tc.tile_pool(name="ty_psum", bufs=2, space="PSUM") as typ,
                tc.tile_pool(name="o_psum", bufs=2, space="PSUM") as opp,
                tc.tile_pool(name="o_st", bufs=3) as ost,
            ):
                for ci in range(NC):
                    ty = typ.tile([64, HPC, 128], F32, tag="ty")
                    for h in range(HPC):
                        nc.tensor.transpose(
                            ty[:, h, :], yt[:, ci, h, :], ident[:])
                    yst = ost.tile([64, HPC, 128], BF16, tag="yst")
                    nc.vector.tensor_copy(yst[:], ty[:])
                    po = opp.tile([128, 256], F32, tag="po")
                    for h in range(HPC):
                        nc.tensor.matmul(
                            po[:], yst[:, h, :], wo_s[:, h, :],
                            start=(h == 0), stop=(h == HPC - 1))
                    ob = ost.tile([128, 256], BF16, tag="ob")
                    nc.vector.tensor_copy(ob[:], po[:])
                    nc.sync.dma_start(
                        bounce_in.rearrange("(c p) d -> p c d", p=128)[
                            :, ci, :], ob[:])

        # ---------- ReduceScatter (bf16) over the batch group + output ------
        if not collective:
            nc.sync.dma_start(
                out_d.ap().rearrange("(c p) d -> p c d", p=128),
                bounce_in.rearrange("(c p) d -> p c d", p=128)[
                    :, 0:NC // 4, :])
            return
        G = len(replica_groups[0])
        bounce_out = dram.tile([N // G, D], BF16)
        nc.gpsimd.collective_compute(
            "ReduceScatter", ALU.add, replica_groups=replica_groups,
            ins=[bounce_in.opt()], outs=[bounce_out.opt()])
        nc.sync.dma_start(out_d.ap(), bounce_out[:])


# ----------------------------------------------------------------------------
# host-side entry point
# ----------------------------------------------------------------------------

_CACHED = {}


def _get_compiled(N=4096, n_cores=8, p_dtype=FP8):
    key = (N, n_cores, p_dtype)
    if key not in _CACHED:
        groups = [list(range(g * 4, g * 4 + 4)) for g in range(2)] \
            if n_cores == 8 else [list(range(n_cores))]
        nc = bacc.Bacc("TRN2", target_bir_lowering=False, debug=False,
                       num_devices=n_cores)
        build_kernel(nc, N=N, replica_groups=groups, p_dtype=p_dtype)
        nc.compile()
        _CACHED[key] = nc
    return _CACHED[key]


def make_in_maps(x, Wq, Wk, Wv, Wo, gamma, beta, coeffs, n_cores=8):
    """Shard full inputs into per-core input maps (batch + head-pair)."""
    x = np.asarray(x, np.float32)
    Wq = np.asarray(Wq, np.float32)
    Wk = np.asarray(Wk, np.float32)
    Wv = np.asarray(Wv, np.float32)
    Wo = np.asarray(Wo, np.float32)
    gamma = np.asarray(gamma, np.float32)
    beta = np.asarray(beta, np.float32)
    coeffs = np.asarray(coeffs, np.float32)
    g2 = np.concatenate([gamma, gamma]).reshape(1, 128).copy()
    b2 = np.concatenate([beta, beta]).reshape(1, 128).copy()
    in_maps = []
    for core in range(n_cores):
        b = core // 4 if n_cores == 8 else 0
        hp = core % 4 if n_cores == 8 else core
        cols = slice(hp * 128, (hp + 1) * 128)
        in_maps.append({
            "x": np.ascontiguousarray(x[b]),
            "wq": np.ascontiguousarray(Wq[:, cols]),
            "wk": np.ascontiguousarray(Wk[:, cols]),
            "wv": np.ascontiguousarray(Wv[:, cols]),
            "wo": np.ascontiguousarray(Wo[cols, :]),
            "gamma2": g2,
            "beta2": b2,
            "cf": np.ascontiguousarray(coeffs[2 * hp: 2 * hp + 2].reshape(1, 8)),
        })
    return in_maps


def kernel(x, Wq, Wk, Wv, Wo, gamma, beta, coeffs, trace=False):
    from concourse.bass_utils import run_bass_kernel_spmd

    n_cores = 8
    nc = _get_compiled(4096, n_cores)
    in_maps = make_in_maps(x, Wq, Wk, Wv, Wo, gamma, beta, coeffs, n_cores)
    res = run_bass_kernel_spmd(nc, in_maps, core_ids=list(range(n_cores)),
                               trace=trace)
    # each core returns its ReduceScatter shard: rank k of a batch group
    # holds rows [k*N/4, (k+1)*N/4) of that batch's output
    N = 4096
    out = np.empty((2, N, 256), np.float32)
    for b, cores in enumerate([[0, 1, 2, 3], [4, 5, 6, 7]]):
        for k, c in enumerate(cores):
            shard = np.asarray(res.results[c]["out"]).astype(np.float32)
            out[b, k * (N // 4):(k + 1) * (N // 4)] = shard
    if trace:
        kernel.last_result = res
    return out


# revision 32
# speedup vs baseline: 1.0198x; 1.0002x over previous
"""AGF attention (graph-filter attention) distributed Bass kernel for 8 TRN2 cores.

Sharding: batch x head-pair (data + head parallel). Core i handles batch
b = i//4 and heads {2*(i%4), 2*(i%4)+1}. Each core computes its partial
output projection (summed over its 2 heads); a bf16 ReduceScatter over the
4 cores of each batch produces row shards of that batch's [N, D] output,
which the host concatenates.

v2 design (per core):
  - P = exp(S^T/8) for the CURRENT head is kept RESIDENT in SBUF
    ([128, NC, N] fp8 = 128 KB/partition) -- no HBM spill/reload. The three
    graph-filter applications read P straight from SBUF.
  - Production: S^T chunks via bf16 matmuls (KT chunk stationary) into
    [128, 2048] PSUM tiles (4 banks x 2 bufs), exp'd by ACT directly into
    P_sb as fp8. ACT is the kernel's bottleneck engine (~250 us of exp).
  - Applications: fp8 DoubleRow matmuls, stationary [t*16 | 1] (TW=80),
    moving P pairs, accumulate u^T = [16*A_u t | r]^T in a [80, NB, 512]
    PSUM tile (8 banks). u^T -> bf16 -> PE-transposed back to natural
    layout in packs of 8 chunks per PSUM bank; epilogue (normalize by 1/r,
    t-requantize to fp8, y accumulation) is BATCHED into a handful of
    full-size DVE instructions using stride-0 broadcast APs.
  - Head 1's Q^T/K^T are produced in setup and spilled to DRAM (bf16),
    reloaded into the same SBUF tiles after head 0's production.
  - Output projection: y (natural, f32, both heads) -> PE transpose ->
    po PSUM accumulates BOTH heads -> bf16 -> ReduceScatter over the
    4-core batch group.
"""

import numpy as np

import concourse.mybir as mybir
import concourse.tile as tile
from concourse import bacc
from concourse.bass import ds
from concourse.masks import make_identity

dt = mybir.dt
F32 = dt.float32
BF16 = dt.bfloat16
FP8 = dt.float8e4
AF = mybir.ActivationFunctionType
ALU = mybir.AluOpType
AX = mybir.AxisListType
DR = mybir.MatmulPerfMode.DoubleRow

D = 256      # model dim
DH = 64      # head dim
HPC = 2      # heads per core
LN_EPS = 1e-5
SM_SCALE = 0.125  # 1/sqrt(DH)
TW = 80      # t tile width: 64 t cols + 1 rowsum + pad to 16 (DoubleRow)
T_SCALE = 16.0


def build_kernel(nc, N=4096, replica_groups=((0, 1, 2, 3), (4, 5, 6, 7)),
                 p_dtype=FP8, collective=True):
    NC = N // 128          # 128-row chunks
    NB = N // 512          # 512-col blocks
    KD = D // 128          # 128-deep contraction chunks of the model dim
    replica_groups = [list(g) for g in replica_groups]

    G = len(replica_groups[0]) if collective else 4
    x = nc.dram_tensor("x", [N, D], F32, kind="ExternalInput")
    wq_d = nc.dram_tensor("wq", [D, HPC * DH], F32, kind="ExternalInput")
    wk_d = nc.dram_tensor("wk", [D, HPC * DH], F32, kind="ExternalInput")
    wv_d = nc.dram_tensor("wv", [D, HPC * DH], F32, kind="ExternalInput")
    wo_d = nc.dram_tensor("wo", [HPC * DH, D], F32, kind="ExternalInput")
    gamma_d = nc.dram_tensor("gamma2", [1, HPC * DH], F32, kind="ExternalInput")
    beta_d = nc.dram_tensor("beta2", [1, HPC * DH], F32, kind="ExternalInput")
    cf_d = nc.dram_tensor("cf", [1, HPC * 4], F32, kind="ExternalInput")
    out_d = nc.dram_tensor("out", [N // G, D], BF16, kind="ExternalOutput")

    with tile.TileContext(nc) as tc:
        _body(nc, tc, N, NC, NB, KD, replica_groups,
              x, wq_d, wk_d, wv_d, wo_d, gamma_d, beta_d, cf_d, out_d,
              collective)
    return nc


def _body(nc, tc, N, NC, NB, KD, replica_groups,
          x, wq_d, wk_d, wv_d, wo_d, gamma_d, beta_d, cf_d, out_d,
          collective=True):
    with (
        tc.tile_pool(name="persist", bufs=1) as pp,
        tc.tile_pool(name="dram", bufs=1, space="DRAM") as dram,
    ):
        # ---------- constants ----------
        ident = pp.tile([128, 128], F32)
        make_identity(nc, ident)
        ident_b = pp.tile([128, 128], BF16)
        nc.vector.tensor_copy(ident_b[:], ident[:])

        c1 = pp.tile([1, 8], F32)
        nc.sync.dma_start(c1[:], cf_d.ap())
        cbc = pp.tile([128, 8], F32)
        nc.gpsimd.partition_broadcast(cbc[:], c1[:])
        cbc16 = pp.tile([128, 8], F32)
        nc.vector.tensor_scalar_mul(cbc16[:], cbc[:], 1.0 / T_SCALE)

        # ---------- persistent work tiles ----------
        QT = pp.tile([64, N], BF16)     # current head's Q^T
        KT = pp.tile([64, N], BF16)     # current head's K^T
        vones = pp.tile([128, NC, HPC, TW], FP8)   # [v_ln | 1] app-1 lhs
        nc.vector.memset(vones[:, :, :, 64:TW], 1.0)
        vln = pp.tile([128, NC, HPC, 64], BF16)    # LayerNorm'd v
        tq = pp.tile([128, NC, TW], FP8)           # [16*t_k | 1] app lhs
        nc.vector.memset(tq[:, :, 64:TW], 1.0)
        uSt = pp.tile([128, NC, TW], BF16)         # transposed-back u chunks
        rinv = pp.tile([128, NC, 1], F32)          # 1/rowsum per (n-chunk)
        yt = pp.tile([128, NC, HPC, 64], F32)      # y, natural, both heads

        wo_s = pp.tile([64, HPC, 256], BF16)
        qk_dr = dram.tile([2, 64, N], BF16)        # head-1 Q^T/K^T spill
        bounce_in = dram.tile([N, D], BF16)

        # ================= setup =================
        with tc.tile_pool(name="setup", bufs=1) as sp:
            # weights -> SBUF bf16
            wst = sp.tile([128, 3, KD, 128], F32)
            nc.sync.dma_start(
                wst[:, 0], wq_d.ap().rearrange("(o p) m -> p o m", p=128))
            nc.sync.dma_start(
                wst[:, 1], wk_d.ap().rearrange("(o p) m -> p o m", p=128))
            nc.scalar.dma_start(
                wst[:, 2], wv_d.ap().rearrange("(o p) m -> p o m", p=128))
            wsb = sp.tile([128, 3, KD, 128], BF16)
            nc.vector.tensor_copy(wsb[:], wst[:])
            wq_s, wk_s, wv_s = wsb[:, 0], wsb[:, 1], wsb[:, 2]

            wo_f = sp.tile([64, HPC, 256], F32)
            nc.scalar.dma_start(
                wo_f[:], wo_d.ap().rearrange("(h d) m -> d h m", h=HPC))
            nc.vector.tensor_copy(wo_s[:], wo_f[:])

            g1 = sp.tile([1, 128], F32)
            nc.sync.dma_start(g1[:], gamma_d.ap())
            gbc = sp.tile([128, 1, HPC, 64], F32)
            nc.gpsimd.partition_broadcast(
                gbc.rearrange("p o a b -> p (o a b)"), g1[:])
            b1 = sp.tile([1, 128], F32)
            nc.sync.dma_start(b1[:], beta_d.ap())
            bbc = sp.tile([128, 1, HPC, 64], F32)
            nc.gpsimd.partition_broadcast(
                bbc.rearrange("p o a b -> p (o a b)"), b1[:])

            # x -> x^T via PE transposes (low latency; no DRAM bounce)
            xT = sp.tile([128, KD, N], BF16)
            with (
                tc.tile_pool(name="xsetup", bufs=3) as xp,
                tc.tile_pool(name="xt_psum", bufs=2, space="PSUM") as xtp,
            ):
                for cg in range(NC // 2):
                    if cg % 2 == 0:
                        xf = xp.tile([128, 4, D], F32, tag="xf")
                        nc.sync.dma_start(
                            xf[:],
                            x.ap().rearrange("(o p) d -> p o d", p=128)[
                                :, ds(cg * 2, 4), :])
                    tps = xtp.tile([128, 2, KD, 128], F32, tag="tx")
                    for j in range(2):
                        for kd in range(KD):
                            nc.tensor.transpose(
                                tps[:, j, kd, :],
                                xf[:, (cg % 2) * 2 + j, ds(kd * 128, 128)],
                                ident[:])
                    # ACT is idle through setup; keep copies off the DVE
                    # critical chain that gates the LayerNorm activations
                    nc.scalar.activation(
                        xT[:, :, ds(cg * 256, 256)].rearrange(
                            "p k (j c) -> p j k c", j=2), tps[:], AF.Copy)

            # ---- Q^T/K^T (head 0 -> SBUF, head 1 -> DRAM) + V/LN ----
            vsb = sp.tile([128, NC, HPC, 64], BF16)
            s1 = sp.tile([128, NC, HPC], F32)
            sqs = sp.tile([128, NC, HPC, 64], BF16)
            s2 = sp.tile([128, NC, HPC], F32)
            mu = sp.tile([128, NC, HPC, 1], F32)
            var = sp.tile([128, NC, HPC, 1], F32)
            rstd = sp.tile([128, NC, HPC, 1], F32)
            with (
                tc.tile_pool(name="qk_psum", bufs=2, space="PSUM") as qpp,
                tc.tile_pool(name="v_psum", bufs=2, space="PSUM") as vpp,
                tc.tile_pool(name="qk_st", bufs=2) as qst,
            ):
                def emit_qk(h):
                    for qi, w_s in ((0, wq_s), (1, wk_s)):
                        for nb in range(N // 1024):
                            ps = qpp.tile([64, 1024], F32, tag="psq")
                            for s in range(2):
                                for kd in range(KD):
                                    nc.tensor.matmul(
                                        ps[:, ds(s * 512, 512)],
                                        w_s[:, kd, ds(h * 64, 64)],
                                        xT[:, kd,
                                           ds(nb * 1024 + s * 512, 512)],
                                        start=(kd == 0), stop=(kd == KD - 1))
                            if h == 0:
                                dst = QT if qi == 0 else KT
                                nc.scalar.activation(
                                    dst[:, ds(nb * 1024, 1024)], ps[:],
                                    AF.Copy)
                            else:
                                stg = qst.tile([64, 1024], BF16, tag="stg")
                                nc.vector.tensor_copy(stg[:], ps[:])
                                nc.scalar.dma_start(
                                    qk_dr[qi, :, ds(nb * 1024, 1024)], stg[:])

                emit_qk(0)
                # V projection: 4 chunks per PSUM bank
                for cg in range(NC // 4):
                    vps = vpp.tile([128, 4, 128], F32, tag="vps")
                    for j in range(4):
                        for kd in range(KD):
                            nc.tensor.matmul(
                                vps[:, j, :],
                                xT[:, kd, ds((cg * 4 + j) * 128, 128)],
                                wv_s[:, kd, :],
                                start=(kd == 0), stop=(kd == KD - 1))
                    nc.vector.tensor_copy(
                        vsb[:, ds(cg * 4, 4), :, :],
                        vps[:].rearrange("p j (h d) -> p j h d", h=HPC))
                emit_qk(1)

            # ---- batched LayerNorm over dim_head ----
            nc.vector.tensor_reduce(
                s1.rearrange("p a b -> p (a b)"), vsb[:], axis=AX.X,
                op=ALU.add)
            nc.vector.tensor_tensor(sqs[:], vsb[:], vsb[:], ALU.mult)
            nc.vector.tensor_reduce(
                s2.rearrange("p a b -> p (a b)"), sqs[:], axis=AX.X,
                op=ALU.add)
            muf = mu.rearrange("p a b c -> p (a b c)")
            varf = var.rearrange("p a b c -> p (a b c)")
            s1f = s1.rearrange("p a b -> p (a b)")
            s2f = s2.rearrange("p a b -> p (a b)")
            nc.vector.tensor_scalar_mul(muf, s1f, 1.0 / 64.0)
            # var = s2/64 - mu^2   (as (s2*(1/64) - mu) ... need mu^2)
            nc.vector.scalar_tensor_tensor(
                varf, muf, -1.0, muf, ALU.mult, ALU.mult)   # -mu^2
            nc.vector.scalar_tensor_tensor(
                varf, s2f, 1.0 / 64.0, varf, ALU.mult, ALU.add)
            nc.vector.tensor_scalar_add(varf, varf, LN_EPS)
            # rstd = exp(-0.5 * ln(var + eps))
            nc.scalar.activation(varf, varf, AF.Ln)
            nc.scalar.activation(
                rstd.rearrange("p a b c -> p (a b c)"), varf,
                AF.Exp, scale=-0.5)
            # vln = (vsb - mu) * rstd * gamma + beta   (broadcast APs)
            mu_b = mu[:].broadcast_to([128, NC, HPC, 64])
            rstd_b = rstd[:].broadcast_to([128, NC, HPC, 64])
            gb = gbc[:].broadcast_to([128, NC, HPC, 64])
            bb = bbc[:].broadcast_to([128, NC, HPC, 64])
            nc.vector.tensor_tensor(vln[:], vsb[:], mu_b, ALU.subtract)
            nc.vector.tensor_tensor(vln[:], vln[:], rstd_b, ALU.mult)
            nc.vector.tensor_tensor(vln[:], vln[:], gb, ALU.mult)
            nc.vector.tensor_tensor(vln[:], vln[:], bb, ALU.add)
            nc.vector.tensor_copy(vones[:, :, :, 0:64], vln[:])

        # ================= main: per-head production + applications =========
        NSPILL = 20                 # head-1 P chunks spilled to DRAM
        with tc.tile_pool(name="pmain", bufs=1) as pm:
            P_sb = pm.tile([128, NC, N], FP8)
            P_dr = dram.tile([NSPILL, 128, N], FP8)

            def app_lhs(h, app, mp):
                return (vones[:, ds(2 * mp, 2), h, :] if app == 1
                        else tq[:, ds(2 * mp, 2), :])

            def emit_app_mms(h, app, apo, uTs, nbp):
                """acc matmuls + copy-out, in NB//nbp n-passes. Yields
                after every mp-group so prod-h1 can interleave."""
                for ip in range(NB // nbp):
                    acc = apo.tile([TW, nbp, 512], F32, tag="acc")
                    for mp in range(NC // 2):
                        lhs = app_lhs(h, app, mp)
                        for j in range(nbp):
                            nc.tensor.matmul(
                                acc[:, j, :], lhs,
                                P_sb[:, ds(2 * mp, 2),
                                     ds((ip * nbp + j) * 512, 512)],
                                start=(mp == 0), stop=(mp == NC // 2 - 1),
                                perf_mode=DR)
                        yield
                    nc.vector.tensor_copy(
                        uTs.rearrange("p (b n) -> p b n", n=512)[
                            :, ds(ip * nbp, nbp), :], acc[:])
                    yield

            def emit_app_tail(h, app, trp, uTs):
                """transpose back (8 chunks/bank) + batched epilogue."""
                for pk in range(NC // 8):
                    tp = trp.tile([128, 8, TW], BF16, tag="tp")
                    for j in range(8):
                        nc.tensor.transpose(
                            tp[:, j, :],
                            uTs[:, ds((pk * 8 + j) * 128, 128)],
                            ident_b[0:TW, 0:TW])
                    nc.vector.tensor_copy(uSt[:, ds(pk * 8, 8), :], tp[:])
                    yield
                if app == 1:
                    nc.vector.reciprocal(rinv[:], uSt[:, :, 64:65])
                rb = rinv[:].broadcast_to([128, NC, 64])
                # tq = (uSt * scale) * (1/r)  [fp8, feeds next app]
                nc.vector.scalar_tensor_tensor(
                    tq[:, :, 0:64], uSt[:, :, 0:64],
                    T_SCALE if app == 1 else 1.0, rb, ALU.mult, ALU.mult)
                yh = yt[:, :, h, :]
                if app == 1:
                    nc.vector.tensor_scalar_mul(
                        yh, vln[:, :, h, :], cbc[:, ds(h * 4, 1)])
                # y += (c_k/16) * tq
                nc.vector.scalar_tensor_tensor(
                    yh, tq[:, :, 0:64], cbc16[:, ds(h * 4 + app, 1)],
                    yh, ALU.mult, ALU.add)
                yield

            # ---- head 0 production: full SBUF residency, 8-bank PSUM ----
            with tc.tile_pool(name="prod_psum", bufs=2, space="PSUM") as ppp:
                for mc in range(NC):
                    for half in range(2):
                        ps = ppp.tile([128, 2048], F32, tag="s")
                        for q in range(4):
                            nc.tensor.matmul(
                                ps[:, ds(q * 512, 512)],
                                KT[:, ds(mc * 128, 128)],
                                QT[:, ds(half * 2048 + q * 512, 512)],
                                start=True, stop=True)
                        nc.scalar.activation(
                            P_sb[:, mc, ds(half * 2048, 2048)], ps[:],
                            AF.Exp, scale=SM_SCALE)

            # head-1 Q^T/K^T reload (waits on prod-h0's last reads)
            nc.sync.dma_start(QT[:], qk_dr[0])
            nc.sync.dma_start(KT[:], qk_dr[1])

            # ---- overlap: apps h0 (PE/DVE) || production h1 (ACT) ----
            # PSUM: prod 2x[128,1024]=4 banks, acc [80,2,512]=2, tr 2.
            with (
                tc.tile_pool(name="ov_prod", bufs=2, space="PSUM") as ovp,
                tc.tile_pool(name="ov_acc", bufs=1, space="PSUM") as apo2,
                tc.tile_pool(name="ov_tr", bufs=2, space="PSUM") as trp2,
                tc.tile_pool(name="ov_sb", bufs=1) as ovs,
                tc.tile_pool(name="spill_sb", bufs=2) as sps,
            ):
                uTs0 = ovs.tile([TW, N], BF16)

                def prod1_unit(mc, qtr, stg):
                    """one quarter-chunk: 2 S-matmuls + exp."""
                    ps = ovp.tile([128, 1024], F32, tag="s1")
                    for q in range(2):
                        nc.tensor.matmul(
                            ps[:, ds(q * 512, 512)],
                            KT[:, ds(mc * 128, 128)],
                            QT[:, ds(qtr * 1024 + q * 512, 512)],
                            start=True, stop=True)
                    dst = (stg[:, ds(qtr * 1024, 1024)] if stg is not None
                           else P_sb[:, mc, ds(qtr * 1024, 1024)])
                    nc.scalar.activation(dst, ps[:], AF.Exp, scale=SM_SCALE)

                def prod1_spill_units():
                    for mc in range(NSPILL):
                        stg = sps.tile([128, N], FP8, tag="pt")
                        for qtr in range(4):
                            prod1_unit(mc, qtr, stg)
                            yield
                        nc.scalar.dma_start(P_dr[mc], stg[:])

                def apps0_units():
                    for app in range(1, 4):
                        yield from emit_app_mms(0, app, apo2, uTs0, nbp=2)
                        yield from emit_app_tail(0, app, trp2, uTs0)

                gp = prod1_spill_units()
                ga = apps0_units()
                done_p = done_a = False
                while not (done_p and done_a):
                    if not done_p:
                        done_p = next(gp, "end") == "end"
                    if not done_a:
                        for _ in range(3):
                            if next(ga, "end") == "end":
                                done_a = True
                                break
                # reload spilled chunks into P_sb (slots free after apps
                # h0; DMAs run during prod-h1's direct tail)
                for p in range(NSPILL // 2):
                    nc.sync.dma_start(
                        P_sb[:, ds(2 * p, 2), :],
                        P_dr[ds(2 * p, 2)].rearrange("c p n -> p c n"))
                # direct-to-SBUF tail of prod h1 (slots freed by apps h0)
                for mc in range(NSPILL, NC):
                    for qtr in range(4):
                        prod1_unit(mc, qtr, None)
                # app1 of head 1: 2-bank accumulator so its early pairs can
                # run under prod-h1's trailing exps
                for _ in emit_app_mms(1, 1, apo2, uTs0, nbp=2):
                    pass
                for _ in emit_app_tail(1, 1, trp2, uTs0):
                    pass

            # ---- apps 2,3 of head 1 (full 8-bank accumulator) ----
            for app in range(2, 4):
                with tc.tile_pool(name="uts", bufs=1) as up:
                    uTs = up.tile([TW, N], BF16)
                    with tc.tile_pool(name="acc_psum", bufs=1,
                                      space="PSUM") as apo:
                        for _ in emit_app_mms(1, app, apo, uTs, nbp=NB):
                            pass
                    with tc.tile_pool(name="tr_psum", bufs=2,
                                      space="PSUM") as trp:
                        for _ in emit_app_tail(1, app, trp, uTs):
                            pass

            # ---- output projection (both heads fused in PSUM) ----
            with (
                tc.tile_pool(name="ty_psum", bufs=2, space="PSUM") as typ,
                tc.tile_pool(name="o_psum", bufs=2, space="PSUM") as opp,
                tc.tile_pool(name="o_st", bufs=3) as ost,
            ):
                for ci in range(NC):
                    ty = typ.tile([64, HPC, 128], F32, tag="ty")
                    for h in range(HPC):
                        nc.tensor.transpose(
                            ty[:, h, :], yt[:, ci, h, :], ident[:])
                    yst = ost.tile([64, HPC, 128], BF16, tag="yst")
                    nc.scalar.activation(yst[:], ty[:], AF.Copy)
                    po = opp.tile([128, 256], F32, tag="po")
                    for h in range(HPC):
                        nc.tensor.matmul(
                            po[:], yst[:, h, :], wo_s[:, h, :],
                            start=(h == 0), stop=(h == HPC - 1))
                    ob = ost.tile([128, 256], BF16, tag="ob")
                    nc.vector.tensor_copy(ob[:], po[:])
                    nc.sync.dma_start(
                        bounce_in.rearrange("(c p) d -> p c d", p=128)[
                            :, ci, :], ob[:])

        # ---------- ReduceScatter (bf16) over the batch group + output ------
        if not collective:
            nc.sync.dma_start(
                out_d.ap().rearrange("(c p) d -> p c d", p=128),
                bounce_in.rearrange("(c p) d -> p c d", p=128)[
                    :, 0:NC // 4, :])
            return
        G = len(replica_groups[0])
        bounce_out = dram.tile([N // G, D], BF16)
        nc.gpsimd.collective_compute(
            "ReduceScatter", ALU.add, replica_groups=replica_groups,
            ins=[bounce_in.opt()], outs=[bounce_out.opt()])
        nc.sync.dma_start(out_d.ap(), bounce_out[:])


# ----------------------------------------------------------------------------
# host-side entry point
# ----------------------------------------------------------------------------

_CACHED = {}


def _get_compiled(N=4096, n_cores=8, p_dtype=FP8):
    key = (N, n_cores, p_dtype)
    if key not in _CACHED:
        groups = [list(range(g * 4, g * 4 + 4)) for g in range(2)] \
            if n_cores == 8 else [list(range(n_cores))]
        nc = bacc.Bacc("TRN2", target_bir_lowering=False, debug=False,
                       num_devices=n_cores)
        build_kernel(nc, N=N, replica_groups=groups, p_dtype=p_dtype)
        nc.compile()
        _CACHED[key] = nc
    return _CACHED[key]


def make_in_maps(x, Wq, Wk, Wv, Wo, gamma, beta, coeffs, n_cores=8):
    """Shard full inputs into per-core input maps (batch + head-pair)."""
    x = np.asarray(x, np.float32)
    Wq = np.asarray(Wq, np.float32)
    Wk = np.asarray(Wk, np.float32)
    Wv = np.asarray(Wv, np.float32)
    Wo = np.asarray(Wo, np.float32)
    gamma = np.asarray(gamma, np.float32)
    beta = np.asarray(beta, np.float32)
    coeffs = np.asarray(coeffs, np.float32)
    g2 = np.concatenate([gamma, gamma]).reshape(1, 128).copy()
    b2 = np.concatenate([beta, beta]).reshape(1, 128).copy()
    in_maps = []
    for core in range(n_cores):
        b = core // 4 if n_cores == 8 else 0
        hp = core % 4 if n_cores == 8 else core
        cols = slice(hp * 128, (hp + 1) * 128)
        in_maps.append({
            "x": np.ascontiguousarray(x[b]),
            "wq": np.ascontiguousarray(Wq[:, cols]),
            "wk": np.ascontiguousarray(Wk[:, cols]),
            "wv": np.ascontiguousarray(Wv[:, cols]),
            "wo": np.ascontiguousarray(Wo[cols, :]),
            "gamma2": g2,
            "beta2": b2,
            "cf": np.ascontiguousarray(coeffs[2 * hp: 2 * hp + 2].reshape(1, 8)),
        })
    return in_maps


def kernel(x, Wq, Wk, Wv, Wo, gamma, beta, coeffs, trace=False):
    from concourse.bass_utils import run_bass_kernel_spmd

    n_cores = 8
    nc = _get_compiled(4096, n_cores)
    in_maps = make_in_maps(x, Wq, Wk, Wv, Wo, gamma, beta, coeffs, n_cores)
    res = run_bass_kernel_spmd(nc, in_maps, core_ids=list(range(n_cores)),
                               trace=trace)
    # each core returns its ReduceScatter shard: rank k of a batch group
    # holds rows [k*N/4, (k+1)*N/4) of that batch's output
    N = 4096
    out = np.empty((2, N, 256), np.float32)
    for b, cores in enumerate([[0, 1, 2, 3], [4, 5, 6, 7]]):
        for k, c in enumerate(cores):
            shard = np.asarray(res.results[c]["out"]).astype(np.float32)
            out[b, k * (N // 4):(k + 1) * (N // 4)] = shard
    if trace:
        kernel.last_result = res
    return out
